# revision 56
# baseline (speedup 1.0000x reference)
"""Trainium2 Bass kernel: dense transformer block (B=4, T=2048, D=1024, F=4096).

Sharding: 8 NeuronCores = data-parallel over batch (4) x causal-balanced
query-half (2). Core (b, h) computes output tokens
  h==0: [0:512) + [1536:2048)      h==1: [512:1536)
of batch element b; k/v are recomputed for all T on each core (no
collectives).

All large GEMMs run as fp8e4 DoubleRow matmuls (K=256 per instruction, 2x
the fp8 rate). LayerNorm is folded into the GEMM pipeline instead of being
materialized:
  - per-token stats via token-major N=1 DoubleRow matmuls (tokens land on
    PSUM partitions, so stats come out pre-columnized; sum/sumsq operands
    x8/x8^2 are host-prepared fp8),
  - the -mu*colsum(W) mean correction enters each qkv PSUM group as a
    rank-1 bf16 matmul (rows bounced column->DRAM->row),
  - rstd is applied at eviction: per-partition scalar for token-major v,
    folded into the softmax Exp scale for k, broadcast-multiply for q,
  - k bias is dropped entirely (softmax is invariant to per-query logit
    constants), v bias folds into the proj bias (softmax rows sum to 1),
    and proj bias folds into the host-prepared residual.
Causal masking and per-core extent kill use -1e9 step matrices accumulated
into the logit PSUM groups; the step data is per-core, the program uniform.

Schedule: per-super v runs before k so the mu-row DRAM bounce hides under
the v matmuls; attention emits logits/exp for both slots before the y/proj
passes so each slot's softmax-denominator bounce overlaps the other slot's
matmuls; the LN2 stat/normalize chain for token-half th is emitted right
after slot th's proj evictions so it overlaps the remaining attention.
"""

import os
import sys

import numpy as np
import ml_dtypes
from contextlib import ExitStack

if "/opt/trn_rl_repo" not in sys.path:  # defensive; normally on PYTHONPATH
    sys.path.append("/opt/trn_rl_repo")

import concourse.bass as bass
import concourse.tile as tile
from concourse import bacc, mybir
from concourse.bass_utils import run_bass_kernel_spmd

P = 128
D = 1024
F = 4096
T = 2048
TQ = 1024            # query tokens per core
W = 512              # matmul moving free dim / token superblock
DC = D // P          # 8 feature chunks
FC = F // P          # 32 ff feature chunks
NSS = T // W         # 4 token superblocks
EXT = (8, 16)        # attention key-chunk extent per query slot
NCORES = 8
EPS = 1e-5
F32 = mybir.dt.float32
BF16 = mybir.dt.bfloat16
FP8 = mybir.dt.float8e4
NPBF16 = ml_dtypes.bfloat16
NPFP8 = ml_dtypes.float8_e4m3
AF = mybir.ActivationFunctionType
OP = mybir.AluOpType
DR = mybir.MatmulPerfMode.DoubleRow

LAST_RESULT = None  # BassKernelResults of the most recent run (for test harness)


def build_program(dbg=False):
    nc = bacc.Bacc(None, target_bir_lowering=False, debug=False)
    dbg_t = {}

    def dbg_tensor(name, shape, dt):
        if dbg:
            dbg_t[name] = nc.dram_tensor("dbg_" + name, shape, dt,
                                         kind="ExternalOutput")

    def dbg_dump(name, ap):
        if dbg:
            nc.sync.dma_start(out=dbg_t[name][:], in_=ap)

    dbg_tensor("murow", [1, 2, T], FP8)
    dbg_tensor("stcol", [P, 16, 2], F32)
    dbg_tensor("rstdc", [P, 16], F32)
    dbg_tensor("k8", [P, DC, T], FP8)
    dbg_tensor("v8", [P, 16, D], FP8)
    dbg_tensor("q8", [P, DC, TQ], FP8)
    dbg_tensor("dcol", [2, P, 4, 2], F32)
    dbg_tensor("y8", [2, P, DC, W], FP8)
    dbg_tensor("x2", [P, DC, TQ], F32)
    dbg_tensor("h28", [2, P, DC, W], FP8)
    dbg_tensor("ae0", [2, P, 2, W], FP8)

    x8 = nc.dram_tensor("x8", [P, DC, T], FP8, kind="ExternalInput")
    xsq8 = nc.dram_tensor("xsq8", [P, DC, T], FP8, kind="ExternalInput")
    xq8 = nc.dram_tensor("xq8", [P, DC, TQ], FP8, kind="ExternalInput")
    xqsq8 = nc.dram_tensor("xqsq8", [P, DC, TQ], FP8, kind="ExternalInput")
    xqr = nc.dram_tensor("xqr", [P, DC, TQ], BF16, kind="ExternalInput")
    wk8 = nc.dram_tensor("wk8", [P, DC, D], FP8, kind="ExternalInput")
    wv8 = nc.dram_tensor("wv8", [P, DC, D], FP8, kind="ExternalInput")
    wq8 = nc.dram_tensor("wq8", [P, DC, D], FP8, kind="ExternalInput")
    wp8 = nc.dram_tensor("wp8", [P, DC, D], FP8, kind="ExternalInput")
    w18 = nc.dram_tensor("w18", [P, DC, F], FP8, kind="ExternalInput")
    w28 = nc.dram_tensor("w28", [P, FC, D], FP8, kind="ExternalInput")
    wsk16 = nc.dram_tensor("wsk16", [1, 2, D], FP8, kind="ExternalInput")
    wqb = nc.dram_tensor("wqb", [2, 2, D], FP8, kind="ExternalInput")
    wsv64 = nc.dram_tensor("wsv64", [1, 2, D], FP8, kind="ExternalInput")
    b1c = nc.dram_tensor("b1c", [P, FC], F32, kind="ExternalInput")
    b2r = nc.dram_tensor("b2r", [1, 2, D], FP8, kind="ExternalInput")
    idm = nc.dram_tensor("idm", [P, P], BF16, kind="ExternalInput")
    steps = nc.dram_tensor("steps", [16, P, W], BF16, kind="ExternalInput")
    xo = nc.dram_tensor("xo", [DC, P, TQ], F32, kind="ExternalOutput")

    with tile.TileContext(nc) as tc, ExitStack() as ctx:
        const = ctx.enter_context(tc.tile_pool(name="const", bufs=1))
        stat = ctx.enter_context(tc.tile_pool(name="stat", bufs=1))
        colp = ctx.enter_context(tc.tile_pool(name="colp", bufs=2))
        rbsp = ctx.enter_context(tc.tile_pool(name="rbsp", bufs=2))
        pstat = ctx.enter_context(tc.tile_pool(name="pstat", bufs=1, space="PSUM"))
        pbc = ctx.enter_context(tc.tile_pool(name="pbc", bufs=2, space="PSUM"))
        pmain = ctx.enter_context(tc.tile_pool(name="pmain", bufs=4, space="PSUM"))
        dramp = ctx.enter_context(tc.tile_pool(name="dram", bufs=1, space="DRAM"))
        # LN2-prep pools live in the outer scope: the prep for token-half th
        # is emitted inside the attention phase (after slot th's proj)
        x28p = ctx.enter_context(tc.tile_pool(name="x28p", bufs=1))
        h2p = ctx.enter_context(tc.tile_pool(name="h2p", bufs=2))
        l2p = ctx.enter_context(tc.tile_pool(name="l2p", bufs=1))
        r2sp = ctx.enter_context(tc.tile_pool(name="r2sp", bufs=2))

        ones8 = const.tile([P, 2, 1], FP8, tag="ones8")
        nc.vector.memset(ones8[:], 1.0)
        ones_row = const.tile([1, P], BF16, tag="ones_row")
        nc.vector.memset(ones_row[:], 1.0)
        eps_t = const.tile([P, 1], F32, tag="eps")
        nc.vector.memset(eps_t[:], EPS)
        dum = const.tile([P, 1], F32, tag="dum")
        for fn in (AF.Sqrt, AF.Exp, AF.Relu):  # preload act tables
            nc.scalar.activation(dum[:], eps_t[:], fn)
        wsk_t = const.tile([1, 2, D], FP8, tag="wsk")
        nc.scalar.dma_start(out=wsk_t[:], in_=wsk16[:])
        wqb_t = const.tile([2, 2, D], FP8, tag="wqb")
        nc.scalar.dma_start(out=wqb_t[:], in_=wqb[:])
        wsv_t = const.tile([1, 2, D], FP8, tag="wsv")
        nc.scalar.dma_start(out=wsv_t[:], in_=wsv64[:])
        b1_t = const.tile([P, FC], F32, tag="b1c")
        nc.scalar.dma_start(out=b1_t[:], in_=b1c[:])
        b2_t = const.tile([1, 2, D], FP8, tag="b2r")
        nc.scalar.dma_start(out=b2_t[:], in_=b2r[:])
        ones8w = const.tile([1, 2, W], FP8, tag="ones8w")
        nc.vector.memset(ones8w[:, 0, :], 1.0)
        nc.vector.memset(ones8w[:, 1, :], 0.0)
        idm_t = const.tile([P, P], BF16, tag="idm")
        nc.scalar.dma_start(out=idm_t[:], in_=idm[:])

        # full-T LN1 stats (columnized by construction: [p, ts] = token ts*128+p)
        stcol = stat.tile([P, 16, 2], F32, tag="stcol")
        muc = stat.tile([P, 16], F32, tag="muc")
        sdc = stat.tile([P, 16], F32, tag="sdc")
        rstdc = stat.tile([P, 16], F32, tag="rstdc")
        rexpc = stat.tile([P, 16], F32, tag="rexpc")     # rstd/32768 (exp scale)
        rvc = stat.tile([P, 16], F32, tag="rvc")         # rstd/4 (v evict scale)
        statmv = stat.tile([P, 16, 2], FP8, tag="statmv")  # col0 ones, col1 16*mu*rstd
        nc.vector.memset(statmv[:, :, 0:1], 1.0)
        mun16 = stat.tile([P, 16], FP8, tag="mun16")     # -16*mu
        murow = stat.tile([1, 2, T], FP8, tag="murow")   # -16*mu row + zero plane
        nc.vector.memset(murow[:, 1, :], 0.0)
        dmu = dramp.tile([T], FP8, tag="dmu")

        # fp32 residual stream x2 stays SBUF-resident across phases 2 and 3
        x2p = ctx.enter_context(tc.tile_pool(name="x2p", bufs=1))
        x2 = x2p.tile([P, DC, TQ], F32, tag="x2")

        def stats_block(src8, srcsq8, pcol, j):
            """Token-major sum/sumsq for token slice j -> pcol [P, 2]."""
            for c in range(4):
                nc.tensor.matmul(pcol[:, 0:1],
                                 src8[:, 2 * c:2 * c + 2, j * P:(j + 1) * P],
                                 ones8[:], start=(c == 0), stop=(c == 3),
                                 perf_mode=DR)
            for c in range(4):
                nc.tensor.matmul(pcol[:, 1:2],
                                 srcsq8[:, 2 * c:2 * c + 2, j * P:(j + 1) * P],
                                 ones8[:], start=(c == 0), stop=(c == 3),
                                 perf_mode=DR)

        def col_math(sc_ap, mu_ap, sd_ap, rstd_ap, n):
            """mu/sd/rstd columns from raw sum/sumsq columns sc_ap [P, n, 2]."""
            nc.vector.tensor_scalar(out=mu_ap, in0=sc_ap[:, :, 0],
                                    scalar1=1.0 / D, scalar2=None, op0=OP.mult)
            tmp = colp.tile([P, n], F32, tag="cm")
            nc.vector.tensor_mul(tmp[:], mu_ap, mu_ap)
            var = colp.tile([P, n], F32, tag="cm")
            nc.vector.scalar_tensor_tensor(
                out=var[:], in0=sc_ap[:, :, 1], scalar=1.0 / D,
                in1=tmp[:], op0=OP.mult, op1=OP.subtract)
            nc.scalar.activation(sd_ap, var[:], AF.Sqrt, bias=eps_t[:])
            nc.vector.reciprocal(rstd_ap, sd_ap)

        # q8 outlives kvp (freed before the MLP) -> created below it in the
        # pool stack; w1's pool likewise spans phases 2-3
        wfp = ctx.enter_context(tc.tile_pool(name="wfp", bufs=1))
        qp = ctx.enter_context(tc.tile_pool(name="qp", bufs=1))
        q8 = qp.tile([P, DC, TQ], FP8, tag="q8")
        skv = ctx.enter_context(ExitStack())
        kvp = skv.enter_context(tc.tile_pool(name="kvp", bufs=1))
        k8 = kvp.tile([P, DC, T], FP8, tag="k8")
        v8 = kvp.tile([P, 16, D], FP8, tag="v8")

        # ---- Phase 1: stats -> v, k per superblock; then q ----
        with ExitStack() as p1:
            xp = p1.enter_context(tc.tile_pool(name="xp", bufs=3))
            xsp = p1.enter_context(tc.tile_pool(name="xsp", bufs=1))
            wkvp = p1.enter_context(tc.tile_pool(name="wkvp", bufs=1))
            xqp = p1.enter_context(tc.tile_pool(name="xqp", bufs=1))
            qsp = p1.enter_context(tc.tile_pool(name="qsp", bufs=1))
            wqp = p1.enter_context(tc.tile_pool(name="wqp", bufs=1))
            # super-0 strips first; q inputs + weights stream in parallel on
            # the other queue
            def load_x8strip(s):
                xs = xp.tile([P, DC, W], FP8, tag="x8s")
                nc.sync.dma_start(out=xs[:], in_=x8[:, :, s * W:(s + 1) * W])
                return xs

            def load_xsqstrip(s):
                xqs = xsp.tile([P, DC, W], FP8, tag="xsqs")
                nc.sync.dma_start(out=xqs[:], in_=xsq8[:, :, s * W:(s + 1) * W])
                return xqs

            strips = [load_x8strip(0)]
            sqstrips = [load_xsqstrip(0)]
            wv_t = wkvp.tile([P, DC, D], FP8, tag="wv")
            nc.scalar.dma_start(out=wv_t[:, 0:4], in_=wv8[:, 0:4])
            nc.scalar.dma_start(out=wv_t[:, 4:8], in_=wv8[:, 4:8])
            wk_t = wkvp.tile([P, DC, D], FP8, tag="wk")
            nc.scalar.dma_start(out=wk_t[:, 0:4], in_=wk8[:, 0:4])
            nc.scalar.dma_start(out=wk_t[:, 4:8], in_=wk8[:, 4:8])
            strips.append(load_x8strip(1))
            xq_t = xqp.tile([P, DC, TQ], FP8, tag="xq8")
            xqsq_t = xqp.tile([P, DC, TQ], FP8, tag="xqsq8")
            wq_t = wqp.tile([P, DC, D], FP8, tag="wq")
            qloads = []
            for hh in range(2):
                hsl = slice(hh * 4, hh * 4 + 4)
                qloads += [(xq_t, xq8, hsl), (xqsq_t, xqsq8, hsl)]
            for hh in range(2):
                hsl = slice(hh * 4, hh * 4 + 4)
                qloads.append((wq_t, wq8, hsl))

            def drain_qloads(n):
                while qloads and n > 0:
                    dst, srcd, hsl = qloads.pop(0)
                    nc.scalar.dma_start(out=dst[:, hsl], in_=srcd[:, hsl])
                    n -= 1

            dq = dramp.tile([2, TQ], FP8, tag="dq")
            dqr = dramp.tile([TQ], BF16, tag="dqr")
            for s in range(NSS):
                xs = strips[s]
                xqs = sqstrips[s]
                tsl = slice(s * W, (s + 1) * W)
                jsl = slice(4 * s, 4 * s + 4)
                for j in range(4):
                    pst = pstat.tile([P, 2], F32, tag="pst")
                    stats_block(xs, xqs, pst, j)
                    nc.vector.tensor_copy(stcol[:, 4 * s + j, :], pst[:])
                if s + 1 < NSS:
                    sqstrips.append(load_xsqstrip(s + 1))
                col_math(stcol[:, jsl, :], muc[:, jsl], sdc[:, jsl],
                         rstdc[:, jsl], 4)
                nc.vector.tensor_scalar(out=rexpc[:, jsl], in0=rstdc[:, jsl],
                                        scalar1=1.0 / 32768.0, scalar2=None,
                                        op0=OP.mult)
                nc.vector.tensor_scalar(out=rvc[:, jsl], in0=rstdc[:, jsl],
                                        scalar1=0.25, scalar2=None, op0=OP.mult)
                m2t = colp.tile([P, 4], F32, tag="cm")
                nc.vector.tensor_mul(m2t[:], muc[:, jsl], rstdc[:, jsl])
                nc.vector.tensor_scalar(
                    out=statmv[:, jsl, 1:2],
                    in0=m2t[:].rearrange("p (j o) -> p j o", o=1),
                    scalar1=16.0, scalar2=None, op0=OP.mult)
                nc.vector.tensor_scalar(out=mun16[:, jsl], in0=muc[:, jsl],
                                        scalar1=-16.0, scalar2=None, op0=OP.mult)
                # bounce -16*mu column -> DRAM -> row (k rank-1 rhs); the v
                # matmuls below hide the round trip
                nc.sync.dma_start(
                    out=dmu[tsl].rearrange("(j p) -> p j", p=P),
                    in_=mun16[:, jsl])
                nc.sync.dma_start(
                    out=murow[0:1, 0, tsl],
                    in_=dmu[tsl].rearrange("(o t) -> o t", o=1))

                for j in range(4):
                    ts = 4 * s + j
                    for fh in range(2):
                        pv = pmain.tile([P, W], F32, tag="mm")
                        for c in range(4):
                            nc.tensor.matmul(
                                pv[:],
                                xs[:, 2 * c:2 * c + 2, j * P:(j + 1) * P],
                                wv_t[:, 2 * c:2 * c + 2, fh * W:(fh + 1) * W],
                                start=(c == 0), stop=(c == 3), perf_mode=DR)
                        if fh == 0:
                            nc.vector.tensor_scalar(
                                out=v8[:, ts, fh * W:(fh + 1) * W], in0=pv[:],
                                scalar1=rvc[:, ts:ts + 1], scalar2=None,
                                op0=OP.mult)
                        else:
                            nc.scalar.activation(
                                v8[:, ts, fh * W:(fh + 1) * W], pv[:], AF.Copy,
                                scale=rvc[:, ts:ts + 1])
                for kf in range(DC):
                    pk = pmain.tile([P, W], F32, tag="mm")
                    for c in range(4):
                        nc.tensor.matmul(pk[:],
                                         wk_t[:, 2 * c:2 * c + 2, kf * P:(kf + 1) * P],
                                         xs[:, 2 * c:2 * c + 2, :],
                                         start=(c == 0), stop=False, perf_mode=DR)
                    nc.tensor.matmul(pk[:], wsk_t[:, :, kf * P:(kf + 1) * P],
                                     murow[:, :, tsl], start=False, stop=True,
                                     perf_mode=DR)
                    nc.scalar.activation(k8[:, kf, tsl], pk[:], AF.Copy)
                if s + 2 < NSS:
                    strips.append(load_x8strip(s + 2))
                drain_qloads(2)
                if s == 2:
                    # q stats + bounce early: DMA engines are quiet here
                    qst = qsp.tile([P, 8, 2], F32, tag="qst")
                    qmu = qsp.tile([P, 8], F32, tag="qmu")
                    qsd = qsp.tile([P, 8], F32, tag="qsd")
                    qrstd = qsp.tile([P, 8], F32, tag="qrstd")
                    qcols = qsp.tile([P, 8, 2], FP8, tag="qcols")
                    qrc16 = qsp.tile([P, 8], BF16, tag="qrc16")
                    qrows = qsp.tile([2, 2, TQ], FP8, tag="qrows")
                    nc.vector.memset(qrows[:, 1, :], 0.0)
                    rqrow = qsp.tile([1, TQ], BF16, tag="rqrow")
                    for j in range(8):
                        pst = pstat.tile([P, 2], F32, tag="pst")
                        stats_block(xq_t, xqsq_t, pst, j)
                        nc.vector.tensor_copy(qst[:, j, :], pst[:])
                    col_math(qst[:], qmu[:], qsd[:], qrstd[:], 8)
                    nc.vector.tensor_scalar(
                        out=qcols[:, :, 0:1],
                        in0=qmu[:].rearrange("p (j o) -> p j o", o=1),
                        scalar1=-16.0, scalar2=None, op0=OP.mult)
                    nc.vector.tensor_copy(
                        qcols[:, :, 1:2],
                        qsd[:].rearrange("p (j o) -> p j o", o=1))
                    nc.vector.tensor_copy(qrc16[:], qrstd[:])
                    for r in range(2):
                        nc.sync.dma_start(
                            out=dq[r, :].rearrange("(j p) -> p j", p=P),
                            in_=qcols[:, :, r])
                    nc.sync.dma_start(
                        out=dqr[:].rearrange("(j p) -> p j", p=P), in_=qrc16[:])
                    nc.sync.dma_start(out=qrows[:, 0, :], in_=dq[0:2, :])
                    nc.sync.dma_start(out=rqrow[:],
                                      in_=dqr[:].rearrange("(o t) -> o t", o=1))

            # -- q matmuls for this core's query tokens --
            drain_qloads(6)
            for qs in range(2):
                qsl = slice(qs * W, (qs + 1) * W)
                rb = pbc.tile([P, W], F32, tag="bc")
                nc.tensor.matmul(rb[:], ones_row[:], rqrow[0:1, qsl],
                                 start=True, stop=True)
                rbs = rbsp.tile([P, W], F32, tag="rbs")
                nc.vector.tensor_copy(rbs[:], rb[:])
                for qf in range(DC):
                    pq = pmain.tile([P, W], F32, tag="mm")
                    for c in range(4):
                        nc.tensor.matmul(pq[:],
                                         wq_t[:, 2 * c:2 * c + 2, qf * P:(qf + 1) * P],
                                         xq_t[:, 2 * c:2 * c + 2, qsl],
                                         start=(c == 0), stop=False, perf_mode=DR)
                    nc.tensor.matmul(pq[:], wqb_t[:, :, qf * P:(qf + 1) * P],
                                     qrows[:, :, qsl], start=False, stop=True,
                                     perf_mode=DR)
                    nc.vector.tensor_mul(q8[:, qf, qsl], pq[:], rbs[:])

        dbg_dump("murow", murow[:])
        dbg_dump("stcol", stcol[:])
        dbg_dump("rstdc", rstdc[:])
        dbg_dump("k8", k8[:])
        dbg_dump("v8", v8[:])
        dbg_dump("q8", q8[:])

        h28s = {}

        eprows = {}

        def eprep_pre(th, thl):
            """LN2 stats for token half th: quantize/square (DVE+Pool), stats,
            cols, bounce of (-mu2, rstd2) rows."""
            x28 = x28p.tile([P, DC, W], FP8, tag="x28")
            x2q = x28p.tile([P, DC, W], FP8, tag="x2sq")
            for c in range(DC):
                if c % 2 == 0:
                    nc.scalar.activation(x28[:, c, :], x2[:, c, thl], AF.Copy)
                else:
                    nc.gpsimd.tensor_copy(x28[:, c, :], x2[:, c, thl])
            for c in range(DC):
                if c % 2 == 0:
                    nc.scalar.square(x2q[:, c, :], x28[:, c, :])
                else:
                    nc.gpsimd.tensor_mul(x2q[:, c, :], x28[:, c, :],
                                         x28[:, c, :])
            l2st = l2p.tile([P, 4, 2], F32, tag="l2st")
            for j in range(4):
                pst = pstat.tile([P, 2], F32, tag="pst")
                stats_block(x28, x2q, pst, j)
                nc.vector.tensor_copy(l2st[:, j, :], pst[:])
            l2mu = l2p.tile([P, 4], F32, tag="l2mu")
            l2sd = l2p.tile([P, 4], F32, tag="l2sd")
            l2rstd = l2p.tile([P, 4], F32, tag="l2rstd")
            col_math(l2st[:], l2mu[:], l2sd[:], l2rstd[:], 4)
            l2c16 = l2p.tile([P, 4, 2], BF16, tag="l2c16")
            nc.vector.tensor_scalar(
                out=l2c16[:, :, 0:1],
                in0=l2mu[:].rearrange("p (j o) -> p j o", o=1),
                scalar1=-1.0, scalar2=None, op0=OP.mult)
            nc.vector.tensor_copy(
                l2c16[:, :, 1:2], l2rstd[:].rearrange("p (j o) -> p j o", o=1))
            for r in range(2):
                nc.sync.dma_start(
                    out=dl2[th, r, :].rearrange("(j p) -> p j", p=P),
                    in_=l2c16[:, :, r])
            l2murow = l2p.tile([1, W], BF16, tag="l2murow")
            nc.sync.dma_start(out=l2murow[:], in_=dl2[th, 0:1, :])
            l2rrow = l2p.tile([1, W], BF16, tag="l2rrow")
            nc.sync.dma_start(out=l2rrow[:], in_=dl2[th, 1:2, :])
            eprows[th] = (x28, l2murow, l2rrow)

        def eprep_mm(th):
            """h28 = (x28 - mu2)*rstd2: broadcast rows, then DVE add+mul."""
            x28, l2murow, l2rrow = eprows[th]
            mb = pbc.tile([P, W], F32, tag="bc")
            nc.tensor.matmul(mb[:], ones_row[:], l2murow[:],
                             start=True, stop=True)
            rb2 = pbc.tile([P, W], F32, tag="bc")
            nc.tensor.matmul(rb2[:], ones_row[:], l2rrow[:],
                             start=True, stop=True)
            r2s = r2sp.tile([P, W], BF16, tag="r2s")
            nc.vector.tensor_copy(r2s[:], rb2[:])
            h28 = h2p.tile([P, DC, W], FP8, tag="h28")
            for c in range(DC):
                xc = r2sp.tile([P, W], FP8, tag="x28c")
                nc.vector.tensor_add(xc[:], x28[:, c, :], mb[:])
                nc.gpsimd.tensor_mul(h28[:, c, :], xc[:], r2s[:])
            h28s[th] = h28

        # ---- Phase 2: attention + proj + residual ----
        with ExitStack() as p2:
            stp = p2.enter_context(tc.tile_pool(name="stp", bufs=1))
            wpp = p2.enter_context(tc.tile_pool(name="wpp", bufs=1))
            xrp = p2.enter_context(tc.tile_pool(name="xrp", bufs=1))
            aep = p2.enter_context(tc.tile_pool(name="aep", bufs=12))
            yp = p2.enter_context(tc.tile_pool(name="yp", bufs=2))
            arp = p2.enter_context(tc.tile_pool(name="arp", bufs=2))
            steps_t = stp.tile([P, 16, W], BF16, tag="steps")
            for i in (2, 3, 0, 1):
                nc.scalar.dma_start(
                    out=steps_t[:, 4 * i:4 * i + 4, :],
                    in_=steps[4 * i:4 * i + 4].rearrange("s p w -> p s w"))
            wp_t = wpp.tile([P, DC, D], FP8, tag="wp")
            nc.scalar.dma_start(out=wp_t[:, 0:4], in_=wp8[:, 0:4])
            nc.scalar.dma_start(out=wp_t[:, 4:8], in_=wp8[:, 4:8])
            xqr_t = xrp.tile([P, DC, TQ], BF16, tag="xqr")
            for i in range(4):
                nc.sync.dma_start(out=xqr_t[:, 2 * i:2 * i + 2, :],
                                  in_=xqr[:, 2 * i:2 * i + 2, :])
            w18_t = wfp.tile([P, DC, F], FP8, tag="w18")
            for i in range(8):
                nc.scalar.dma_start(out=w18_t[:, :, i * W:(i + 1) * W],
                                    in_=w18[:, :, i * W:(i + 1) * W])
            dslm = dramp.tile([2, W], FP8, tag="dslm")
            dslr = dramp.tile([2, W], BF16, tag="dslr")
            dl2 = dramp.tile([2, 2, W], BF16, tag="dl2")

            aes_k = {}
            rows_k = {}
            # pass A per slot (1 first): logits+exp, then denominators +
            # row bounce; each slot's bounce overlaps the next slot's logits
            for ka in (1, 0):
                ext = EXT[ka]
                qsl = slice(ka * W, (ka + 1) * W)
                aes = []
                for sc in range(ext):
                    pl = pmain.tile([P, W], F32, tag="mm")
                    for c in range(4):
                        nc.tensor.matmul(pl[:],
                                         k8[:, 2 * c:2 * c + 2, sc * P:(sc + 1) * P],
                                         q8[:, 2 * c:2 * c + 2, qsl],
                                         start=(c == 0),
                                         stop=(c == 3 and not (ka == 0 or sc >= 8)),
                                         perf_mode=DR)
                    if ka == 0 or sc >= 8:
                        nc.tensor.matmul(pl[:], idm_t[:], steps_t[:, sc, :],
                                         start=False, stop=True)
                    i, j = sc // 2, sc % 2
                    if j == 0:
                        ae_t = aep.tile([P, 2, W], FP8, tag="ae")
                        aes.append(ae_t)
                    nc.scalar.activation(aes[i][:, j, :], pl[:], AF.Exp,
                                         scale=rexpc[:, sc:sc + 1])
                aes_k[ka] = aes
                # groups must stay sequential within the shared PSUM bank
                # (interleaving across pairs corrupts them)
                pdt = pstat.tile([P, 4, 2], F32, tag="pd")
                for qs in range(4):
                    for i in range(ext // 2):
                        nc.tensor.matmul(
                            pdt[:, qs, :],
                            aes[i][:, :, qs * P:(qs + 1) * P],
                            statmv[:, 2 * i:2 * i + 2, :],
                            start=(i == 0), stop=(i == ext // 2 - 1),
                            perf_mode=DR)
                dcol = colp.tile([P, 4, 2], F32, tag="dcol")
                nc.vector.tensor_copy(dcol[:], pdt[:])
                if dbg:
                    nc.sync.dma_start(out=dbg_t["dcol"][ka], in_=dcol[:])
                    nc.sync.dma_start(out=dbg_t["ae0"][ka], in_=aes[0][:])
                rcol = colp.tile([P, 4], F32, tag="rcol")
                nc.vector.reciprocal(rcol[:], dcol[:, :, 0])
                nm2c = colp.tile([P, 4], FP8, tag="nm2c")
                nc.vector.tensor_scalar(out=nm2c[:], in0=dcol[:, :, 1],
                                        scalar1=-0.25, scalar2=None, op0=OP.mult)
                rc16 = colp.tile([P, 4], BF16, tag="rc16")
                nc.vector.tensor_copy(rc16[:], rcol[:])
                nc.sync.dma_start(
                    out=dslm[ka, :].rearrange("(j p) -> p j", p=P), in_=nm2c[:])
                nc.sync.dma_start(
                    out=dslr[ka, :].rearrange("(j p) -> p j", p=P), in_=rc16[:])
                nm2row = arp.tile([1, 2, W], FP8, tag="nm2row")
                nc.vector.memset(nm2row[:, 1, :], 0.0)
                nc.sync.dma_start(out=nm2row[0:1, 0, :],
                                  in_=dslm[ka, :].rearrange("(o t) -> o t", o=1))
                rrow = arp.tile([1, W], BF16, tag="rrow")
                nc.sync.dma_start(out=rrow[:],
                                  in_=dslr[ka, :].rearrange("(o t) -> o t", o=1))
                rb = pbc.tile([P, W], F32, tag="bc")
                nc.tensor.matmul(rb[:], ones_row[:], rrow[:],
                                 start=True, stop=True)
                rbs = rbsp.tile([P, W], F32, tag="rbs")
                nc.vector.tensor_copy(rbs[:], rb[:])
                rows_k[ka] = (nm2row, rbs)

            # pass B: per slot (1 then 0): bcast, y, proj, LN2-prep
            for ka in (1, 0):
                ext = EXT[ka]
                qsl = slice(ka * W, (ka + 1) * W)
                aes = aes_k[ka]
                nm2row, rbs = rows_k[ka]
                y8 = yp.tile([P, DC, W], FP8, tag="y8")
                for cc in range(DC):
                    py = pmain.tile([P, W], F32, tag="mm")
                    for i in range(ext // 2):
                        nc.tensor.matmul(py[:],
                                         v8[:, 2 * i:2 * i + 2, cc * P:(cc + 1) * P],
                                         aes[i][:], start=(i == 0), stop=False,
                                         perf_mode=DR)
                    nc.tensor.matmul(py[:], wsv_t[:, :, cc * P:(cc + 1) * P],
                                     nm2row[:], start=False, stop=True,
                                     perf_mode=DR)
                    nc.vector.tensor_mul(y8[:, cc, :], py[:], rbs[:])
                if dbg:
                    nc.sync.dma_start(out=dbg_t["y8"][ka], in_=y8[:])
                for cp in range(DC):
                    pp = pmain.tile([P, W], F32, tag="mm")
                    for c in range(4):
                        nc.tensor.matmul(pp[:],
                                         wp_t[:, 2 * c:2 * c + 2, cp * P:(cp + 1) * P],
                                         y8[:, 2 * c:2 * c + 2, :],
                                         start=(c == 0), stop=(c == 3),
                                         perf_mode=DR)
                    nc.vector.scalar_tensor_tensor(
                        out=x2[:, cp, qsl], in0=pp[:], scalar=1.0 / 256.0,
                        in1=xqr_t[:, cp, qsl], op0=OP.mult, op1=OP.add)
                eprep_pre(ka, qsl)
                if ka == 1:
                    eprep_mm(1)

        dbg_dump("x2", x2[:])
        skv.close()  # release k8/v8 SBUF before the MLP phase

        # ---- Phase 3: MLP + residual ----
        with ExitStack() as p3:
            rfp = p3.enter_context(tc.tile_pool(name="rfp", bufs=2))
            evp = p3.enter_context(tc.tile_pool(name="evp", bufs=2))
            w2p = p3.enter_context(tc.tile_pool(name="w2p", bufs=1))
            w28_t = w2p.tile([P, FC, D], FP8, tag="w28")
            for i in range(8):
                nc.scalar.dma_start(out=w28_t[:, 4 * i:4 * i + 4, :],
                                    in_=w28[:, 4 * i:4 * i + 4, :])
            for th in (1, 0):
                thl = slice(th * W, (th + 1) * W)
                if th == 0:
                    eprep_mm(0)
                h28 = h28s[th]
                if dbg:
                    nc.sync.dma_start(out=dbg_t["h28"][th], in_=h28[:])
                rf = rfp.tile([P, FC, W], FP8, tag="rf")
                for fc in range(FC):
                    pf = pmain.tile([P, W], F32, tag="mm")
                    for c in range(4):
                        nc.tensor.matmul(pf[:],
                                         w18_t[:, 2 * c:2 * c + 2, fc * P:(fc + 1) * P],
                                         h28[:, 2 * c:2 * c + 2, :],
                                         start=(c == 0), stop=(c == 3),
                                         perf_mode=DR)
                    nc.scalar.activation(rf[:, fc, :], pf[:], AF.Relu,
                                         bias=b1_t[:, fc:fc + 1], scale=0.25)
                for cp in range(DC):
                    po = pmain.tile([P, W], F32, tag="mm")
                    for i in range(FC // 2):
                        nc.tensor.matmul(po[:],
                                         w28_t[:, 2 * i:2 * i + 2, cp * P:(cp + 1) * P],
                                         rf[:, 2 * i:2 * i + 2, :],
                                         start=(i == 0), stop=False, perf_mode=DR)
                    nc.tensor.matmul(po[:], b2_t[:, :, cp * P:(cp + 1) * P],
                                     ones8w[:], start=False, stop=True,
                                     perf_mode=DR)
                    ev = evp.tile([P, W], F32, tag="evf")
                    nc.vector.scalar_tensor_tensor(
                        out=ev[:], in0=po[:], scalar=1.0 / 512.0,
                        in1=x2[:, cp, thl], op0=OP.mult, op1=OP.add)
                    nc.sync.dma_start(out=xo[cp, :, thl], in_=ev[:])

    nc.finalize()
    return nc


def _q_idx(h):
    if h == 0:
        return np.concatenate([np.arange(0, W), np.arange(T - W, T)])
    return np.arange(W, T - W)


def _chunk(a):
    """[D, N] -> [P, DC, N] feature-chunked layout ((c p) n -> p c n)."""
    d, n = a.shape
    return np.ascontiguousarray(a.reshape(d // P, P, n).transpose(1, 0, 2))


def _build_steps(h):
    t0s = (0, T - W) if h == 0 else (W, 2 * W)
    m = np.zeros((16, P, W), np.float32)
    for sc in range(16):
        ka = 0 if sc < 8 else 1
        s = sc * P + np.arange(P)[:, None]
        t = t0s[ka] + np.arange(W)[None, :]
        m[sc] = np.where(s <= t, 0.0, -1e9)
    return m.astype(NPBF16)


_cache = {}


def _get_program():
    if "nc" not in _cache:
        _cache["nc"] = build_program()
    return _cache["nc"]


def kernel(**inputs):
    global LAST_RESULT
    f32 = np.float32
    x = np.asarray(inputs["x"], dtype=f32)
    wqkv = np.asarray(inputs["qkv_w"], dtype=f32)
    bqkv = np.asarray(inputs["qkv_b"], dtype=f32)
    wproj = np.asarray(inputs["proj_w"], dtype=f32)
    bproj = np.asarray(inputs["proj_b"], dtype=f32)
    w1 = np.asarray(inputs["ff1_w"], dtype=f32)
    b1 = np.asarray(inputs["ff1_b"], dtype=f32)
    w2 = np.asarray(inputs["ff2_w"], dtype=f32)
    b2 = np.asarray(inputs["ff2_b"], dtype=f32)

    wq8 = (32.0 * wqkv[:, 0:D]).astype(NPFP8)
    wk8 = (32.0 * wqkv[:, D:2 * D]).astype(NPFP8)
    wv8 = (32.0 * wqkv[:, 2 * D:3 * D]).astype(NPFP8)
    wp8 = (32.0 * wproj).astype(NPFP8)
    w18 = (32.0 * w1).astype(NPFP8)
    w28 = (64.0 * w2).astype(NPFP8)
    def zplane(row):
        return np.stack([row, np.zeros_like(row)], axis=-2).astype(NPFP8)

    wsk16 = zplane((wk8.astype(f32).sum(0) / 16.0)[None, :])
    wqb = zplane(np.stack([wq8.astype(f32).sum(0) / 16.0, 32.0 * bqkv[0:D]]))
    wsv64 = zplane((wv8.astype(f32).sum(0) / 16.0)[None, :])
    bv = bqkv[2 * D:3 * D]
    bpp = bproj + bv @ (wp8.astype(f32) / 32.0)
    b1c = np.ascontiguousarray((8.0 * b1).reshape(FC, P).T)
    b2r = zplane((512.0 * b2)[None, :])
    idm = np.eye(P, dtype=f32).astype(NPBF16)
    steps_h = {h: _build_steps(h) for h in (0, 1)}

    shared = dict(
        wq8=_chunk(wq8), wk8=_chunk(wk8), wv8=_chunk(wv8), wp8=_chunk(wp8),
        w18=_chunk(w18), w28=_chunk(w28),
        wsk16=wsk16, wqb=wqb, wsv64=wsv64, b1c=b1c, b2r=b2r, idm=idm,
    )

    in_maps = []
    for core in range(NCORES):
        b, h = core >> 1, core & 1
        xt = np.ascontiguousarray(x[b].T)                  # [D, T]
        x8 = xt.astype(NPFP8)
        xsq8 = (xt ** 2).astype(NPFP8)
        qi = _q_idx(h)
        xq8 = np.ascontiguousarray(x8[:, qi])
        xqsq8 = np.ascontiguousarray(xsq8[:, qi])
        xqr = (xt[:, qi] + bpp[:, None]).astype(NPBF16)
        in_maps.append(dict(
            x8=_chunk(x8), xsq8=_chunk(xsq8), xq8=_chunk(xq8),
            xqsq8=_chunk(xqsq8), xqr=_chunk(xqr), steps=steps_h[h],
            **shared,
        ))

    nc = _get_program()
    trace = os.environ.get("KERNEL_TRACE", "0") == "1"
    res = run_bass_kernel_spmd(nc, in_maps, list(range(NCORES)), trace=trace)
    LAST_RESULT = res

    out = np.empty((4, T, D), f32)
    for core in range(NCORES):
        b, h = core >> 1, core & 1
        xoc = np.asarray(res.results[core]["xo"])          # [DC, P, TQ]
        out[b, _q_idx(h), :] = xoc.transpose(2, 0, 1).reshape(TQ, D)
    return out


if __name__ == "__main__":
    nc = build_program()
    print("program built ok:",
          sum(len(b.instructions) for b in nc.main_func.blocks), "instructions")
